# revision 36
# baseline (speedup 1.0000x reference)
"""DOA pattern loss kernel for Trainium2 (8 NeuronCores, SPMD).

Computes min_r sum_a (possible_phases[r, a] - phases[a])^2 over a
[1_000_000, 32] codebook, returning the scalar min.

Strategy (memory-bound problem; minimize bytes moved):
  - Quantize the codebook to 4 bits on a uniform grid over [0, 2pi):
    q = round(x * 15.5/(2pi)) in [0, 15], two antennas packed per byte
    -> 16 MB shipped to HBM instead of 128 MB fp32 (8x).
  - Shard rows across 8 cores (126976 rows each = 8 octants x 15872
    positions, padded with duplicate rows).  Per-core layout
    cb4[128, 15872] u8: partition p = 16*octant + antenna-pair, free dim
    = row position, so the antenna reduction lands on SBUF partitions
    and DMA lines are contiguous.
  - Device: DVE extracts nibbles (bitwise and / shift, u8->u8), ScalarE
    squares with the -p/s bias folded into the activation (u8 in, exact),
    TensorE sums antenna groups with two accumulating matmuls per
    512-row chunk (lo + hi nibble planes) against an s^2-scaled ones
    matrix, DVE converts each PSUM bank to u8 distances at 0.5
    granularity (saturating at 255), and a partition-strided DMA writes
    the de-replicated per-row distances out (1 MB total D2H).
  - Host: takes the quantized argmin and exactly rescores (in fp32, from
    the original input) every row whose quantized distance is within
    MARGIN of the quantized min.  The 4-bit pass only has to rank rows
    to within the margin; the returned value is exact (the quantization
    noise std per row is ~1.3, MARGIN = 10 is ~8 sigma).
"""

import numpy as np

P = 128
A = 32
OCT = 8          # row-octants stacked on the partition axis
PAIRS = 16       # antenna pairs (= 2 antennas per byte-column)
CHUNK = 512
NCORES = 8

QP4 = 15872      # positions per octant per core = 31 * 512
RC4 = OCT * QP4  # rows per core = 126976

TWO_PI = 2.0 * np.pi
QL4 = 15.5
S4 = TWO_PI / QL4

OUT_SCALE = 2.0  # u8 distance granularity = 0.5
MARGIN = 10.0    # exactly rescore every row within this of the quantized min
BLOCK = 32       # positions per block-min (device output granularity)

_cache: dict = {}


def _widths(qp: int, w: int):
    """DMA tile widths: full w tiles, then a halving taper (short drain)."""
    widths = []
    rem = qp
    while rem > w:
        widths.append(w)
        rem -= w
    while rem > 2 * CHUNK:
        half = ((rem // 2 + CHUNK - 1) // CHUNK) * CHUNK
        widths.append(half)
        rem -= half
    widths.append(rem)
    return widths


def _groups(qp: int = QP4, w: int = 4096):
    """(chunk_base, gch) per PSUM group, matching build_nc4's iteration."""
    groups = []
    cbase = 0
    for wt in _widths(qp, w):
        nch = wt // CHUNK
        for g0 in range(0, nch, 4):
            groups.append((cbase + g0, min(4, nch - g0)))
        cbase += nch
    return groups


def build_nc4(qp: int = QP4, w: int = 4096, sq_w: int = 2048, reps: int = 1,
              xbufs: int = 4, dbufs: int = 3, pbufs: int = 6,
              frac_act: float = 1.0):
    from contextlib import ExitStack

    import concourse.bacc as bacc
    import concourse.tile as tile
    from concourse import mybir

    dt = mybir.dt.float16
    u8 = mybir.dt.uint8
    nc = bacc.Bacc("TRN2", target_bir_lowering=False)

    cb = nc.dram_tensor("cb", [P, qp], u8, kind="ExternalInput")
    neglo = nc.dram_tensor("neglo", [P, 1], mybir.dt.float32, kind="ExternalInput")
    neghi = nc.dram_tensor("neghi", [P, 1], mybir.dt.float32, kind="ExternalInput")
    bmat = nc.dram_tensor("bmat", [P, A], dt, kind="ExternalInput")
    nslots = min(reps, 4)  # timing runs reuse output slots round-robin
    n_groups = 0

    widths = _widths(qp, w)
    offs = []
    o = 0
    for wt in widths:
        assert wt % CHUNK == 0 and wt > 0
        offs.append((o, wt))
        o += wt
        n_groups += (wt // CHUNK + 3) // 4
    nb = CHUNK // BLOCK  # block-mins per chunk (16)
    # one u8 block-min per (group, chunk-slot, octant, block): [32, 16*n_groups]
    outd = nc.dram_tensor(
        "outd", [A, nb * n_groups * nslots], u8, kind="ExternalOutput"
    )

    with tile.TileContext(nc) as tc:
        with ExitStack() as ctx:
            singles = ctx.enter_context(tc.tile_pool(name="singles", bufs=1))
            xpool = ctx.enter_context(tc.tile_pool(name="xin", bufs=xbufs))
            lpool = ctx.enter_context(tc.tile_pool(name="xl", bufs=dbufs))
            hpool = ctx.enter_context(tc.tile_pool(name="xh", bufs=dbufs))
            dlpool = ctx.enter_context(tc.tile_pool(name="d2l", bufs=dbufs))
            dhpool = ctx.enter_context(tc.tile_pool(name="d2h", bufs=dbufs))
            spool = ctx.enter_context(tc.tile_pool(name="stg", bufs=4))
            ppool = ctx.enter_context(tc.tile_pool(name="ps", bufs=pbufs, space="PSUM"))

            neglo_s = singles.tile([P, 1], mybir.dt.float32)
            nc.scalar.dma_start(out=neglo_s[:, :], in_=neglo[:, :])
            neghi_s = singles.tile([P, 1], mybir.dt.float32)
            nc.scalar.dma_start(out=neghi_s[:, :], in_=neghi[:, :])
            b_s = singles.tile([P, A], dt)
            nc.scalar.dma_start(out=b_s[:, :], in_=bmat[:, :])

            BIG = 3.0e38
            acc_a = 0.0
            for rep in range(reps):
                # per-rep block-min accumulator: [128, 16 * n_groups] fp32
                # (negated distances; unwritten partial-group partitions get
                # -BIG so they decode as saturated-far)
                pmin = spool.tile([P, nb * n_groups], mybir.dt.float32, tag="pm")
                nc.vector.memset(pmin[:, :], BIG)
                gidx = 0
                for o, wt in offs:
                    x = xpool.tile([P, w], u8, tag="x")
                    nc.sync.dma_start(out=x[:, :wt], in_=cb[:, o : o + wt])

                    # bitwise TSP can't cast: extract u8 -> u8, squares cast
                    xl = lpool.tile([P, w], u8, tag="xl")
                    xh = hpool.tile([P, w], u8, tag="xh")
                    d2l = dlpool.tile([P, w], dt, tag="dl")
                    d2h = dhpool.tile([P, w], dt, tag="dh")
                    for so in range(0, wt, sq_w):
                        sw = min(sq_w, wt - so)
                        sl = slice(so, so + sw)
                        nc.vector.tensor_scalar(
                            out=xl[:, sl], in0=x[:, sl],
                            scalar1=15, scalar2=0,
                            op0=mybir.AluOpType.bitwise_and,
                            op1=mybir.AluOpType.bitwise_or,
                        )
                        nc.vector.tensor_scalar(
                            out=xh[:, sl], in0=x[:, sl],
                            scalar1=4, scalar2=0,
                            op0=mybir.AluOpType.logical_shift_right,
                            op1=mybir.AluOpType.bitwise_or,
                        )
                        # squares: ACT with the -p/s bias folded in; a slice
                        # may be moved to DVE by frac_act < 1 (ts_add + mul).
                        nch = sw // CHUNK
                        acc_a += nch * frac_act
                        na = min(int(acc_a + 0.5), nch)
                        acc_a -= na
                        aw = so + na * CHUNK
                        if aw > so:
                            nc.scalar.activation(
                                d2l[:, so:aw], xl[:, so:aw],
                                mybir.ActivationFunctionType.Square,
                                bias=neglo_s[:, :], scale=1.0,
                            )
                            nc.scalar.activation(
                                d2h[:, so:aw], xh[:, so:aw],
                                mybir.ActivationFunctionType.Square,
                                bias=neghi_s[:, :], scale=1.0,
                            )
                        if so + sw > aw:
                            el = slice(aw, so + sw)
                            nc.vector.tensor_scalar_add(
                                d2l[:, el], xl[:, el], neglo_s[:, :]
                            )
                            nc.vector.tensor_mul(
                                d2l[:, el], d2l[:, el], d2l[:, el]
                            )
                            nc.vector.tensor_scalar_add(
                                d2h[:, el], xh[:, el], neghi_s[:, :]
                            )
                            nc.vector.tensor_mul(
                                d2h[:, el], d2h[:, el], d2h[:, el]
                            )

                    tile_nch = wt // CHUNK
                    for g0 in range(0, tile_nch, 4):
                        gch = min(4, tile_nch - g0)
                        ps = ppool.tile([P, CHUNK], mybir.dt.float32, tag="ps")
                        for jj in range(gch):
                            c = g0 + jj
                            cs = slice(c * CHUNK, (c + 1) * CHUNK)
                            nc.tensor.matmul(
                                ps[32 * jj : 32 * (jj + 1), :],
                                b_s[:, :],
                                d2l[:, cs],
                                start=True,
                                stop=False,
                                tile_position=(0, 32 * jj),
                            )
                            nc.tensor.matmul(
                                ps[32 * jj : 32 * (jj + 1), :],
                                b_s[:, :],
                                d2h[:, cs],
                                start=False,
                                stop=True,
                                tile_position=(0, 32 * jj),
                            )
                        npart = 32 * gch
                        # windowed min over 32-position blocks: 16 per chunk
                        nc.vector.tensor_reduce(
                            out=pmin[:npart, gidx * nb : (gidx + 1) * nb],
                            in_=ps[:npart, :].rearrange(
                                "p (w b) -> p w b", b=BLOCK
                            ),
                            axis=mybir.AxisListType.X,
                            op=mybir.AluOpType.min,
                        )
                        gidx += 1

                assert gidx == n_groups
                stg = spool.tile([P, nb * n_groups], u8, tag="stg")
                nc.vector.tensor_scalar(
                    out=stg[:, :], in0=pmin[:, :],
                    scalar1=OUT_SCALE, scalar2=255.0,
                    op0=mybir.AluOpType.mult,
                    op1=mybir.AluOpType.min,
                )
                ob = (rep % nslots) * nb * n_groups
                nc.sync.dma_start(
                    out=outd[:, ob : ob + nb * n_groups],
                    in_=stg[0:P:4, :],
                )

    nc.compile()
    return nc


def make_in_maps4(possible_phases: np.ndarray, phases: np.ndarray, qp: int = QP4):
    rc = OCT * qp
    rpad = NCORES * rc
    pp = np.asarray(possible_phases, dtype=np.float32)
    q = np.minimum((pp * (QL4 / TWO_PI) + np.float32(0.5)).astype(np.uint8), 15)
    r = q.shape[0]
    assert rpad >= r and rpad - r <= r, (rpad, r)
    if rpad > r:
        q = np.concatenate([q, q[: rpad - r]], axis=0)  # duplicate-row pad
    packed = (q[:, 0::2] | (q[:, 1::2] << 4)).astype(np.uint8)  # [rpad, 16]

    ph = np.asarray(phases, dtype=np.float32).reshape(A)
    pair = np.arange(P) % PAIRS
    neglo = (-ph[2 * pair] / S4).reshape(P, 1).astype(np.float32)
    neghi = (-ph[2 * pair + 1] / S4).reshape(P, 1).astype(np.float32)
    # B[k, m] = s^2 iff k//16 == m//4 (8 octants, 4x replicated along m)
    bmat = np.float16(S4 * S4) * np.kron(
        np.eye(OCT, dtype=np.float16), np.ones((PAIRS, A // OCT), dtype=np.float16)
    )

    in_maps = []
    for c in range(NCORES):
        shard = packed[c * rc : (c + 1) * rc]  # [rc, 16]
        cbq = np.ascontiguousarray(
            shard.reshape(OCT, qp, PAIRS).transpose(0, 2, 1).reshape(P, qp)
        )
        in_maps.append({"cb": cbq, "neglo": neglo, "neghi": neghi, "bmat": bmat})
    return in_maps


def refine(results, pp: np.ndarray, ph: np.ndarray):
    """Quantized per-block min distances -> exact min via host rescore.

    outd[p2, gi*16 + b] = block-min of octant p2 % 8, chunk cbase[gi] + p2//8,
    positions [b*32, (b+1)*32) -- valid only while p2//8 < gch[gi].
    """
    n_rows = pp.shape[0]
    groups = _groups()
    ng = len(groups)
    nb = CHUNK // BLOCK
    cbase = np.array([g[0] for g in groups])          # [ng]
    gch = np.array([g[1] for g in groups])            # [ng]

    p2 = np.arange(A)
    jj = p2 // OCT                                    # chunk slot in group
    oct_ = p2 % OCT
    valid = jj[:, None] < gch[None, :]                # [32, ng]
    chunk = cbase[None, :] + jj[:, None]              # [32, ng]
    # start row (within a core) for each (p2, gi, b) block
    start = (
        oct_[:, None, None] * QP4
        + chunk[:, :, None] * CHUNK
        + np.arange(nb)[None, None, :] * BLOCK
    )                                                 # [32, ng, nb]
    valid3 = np.broadcast_to(valid[:, :, None], start.shape)

    dmin = np.inf
    dists = []
    for c in range(NCORES):
        od = np.asarray(results[c]["outd"])[:, : ng * nb]
        d = od.astype(np.float32).reshape(A, ng, nb) / OUT_SCALE
        d = np.where(valid3, d, np.inf)
        dists.append(d)
        dmin = min(dmin, d.min())

    rows = []
    for c in range(NCORES):
        sel = np.nonzero(dists[c] <= dmin + MARGIN)
        starts = start[sel] + c * RC4
        if len(starts):
            rows.append((starts[:, None] + np.arange(BLOCK)[None, :]).ravel())
    cand = np.concatenate(rows) if rows else np.arange(min(n_rows, 1024))
    cand = cand[cand < n_rows]  # padded rows duplicate early rows, still in set
    if len(cand) == 0:
        cand = np.arange(min(n_rows, 1024))
    diff = pp[cand] - ph
    return np.float32((diff * diff).sum(1).min())


def kernel(possible_phases: np.ndarray, phases: np.ndarray) -> np.ndarray:
    pp = np.asarray(possible_phases, dtype=np.float32)
    ph = np.asarray(phases, dtype=np.float32)
    if pp.shape != (1_000_000, A) or ph.shape != (A,):
        # safety net for unexpected shapes: exact numpy fallback
        diff = pp - ph.reshape(1, -1)
        return np.float32((diff * diff).sum(1).min())

    from concourse.bass_utils import run_bass_kernel_spmd

    if "nc" not in _cache:
        _cache["nc"] = build_nc4()
    in_maps = make_in_maps4(pp, ph)
    res = run_bass_kernel_spmd(_cache["nc"], in_maps, core_ids=list(range(NCORES)))
    return refine(res.results, pp, ph)
